# revision 22
# baseline (speedup 1.0000x reference)
"""Bass/Tile TRN2 kernel for nn_MultiHeadSelfAttention (B=2, S=2048, D=1024, H=16).

Sharding: 8 cores; core c handles batch b=c//4 and the 4 heads hg=c%4
(e-slice of 256 columns of the fused QKV/out projections).

Per-core device program (SPMD — same NEFF on every core, different data):
  - fp16 everywhere on the PE (1 cycle/row, 1024-wide moving operands,
    fast weight loads); PSUM accumulation is fp32.
  - Q/K projections into transposed layout qT/kT [e, s]; V projection into
    natural layout [s, e] with an appended ones column (gives Z for free).
  - Per (q-half, head): scoresT[k, q] = kT_h^T @ qT_h (d=64 contraction),
    exp on ACT (key-padding mask folded in as a per-partition bias),
    ctx~T[65, q] = vv_h^T @ expT accumulated over k in PSUM (row 64 = Z).
    The PE stream is software-pipelined (attnV trails scores by 2 steps)
    so the PE never stalls on ACT and the HAM clock stays at 2.4 GHz.
  - One batched reciprocal per q-half for all 4 heads' Z rows; 1/Z is
    partition-broadcast on the (otherwise idle) GpSimd engine.
  - attn-mean partial (kt-major): acc[k,q] = sum_h expT_h * (1/Z_h), DVE fp16.
  - out-proj: outT[f, s] partial = Wo-slice chunks^T @ ctxT.
Host: sums the 4 per-batch core partials, transposes, adds bias terms.
"""

import sys

sys.path.insert(0, "/opt/trn_rl_repo")

import numpy as np
import concourse.bass as bass  # noqa: F401
import concourse.mybir as mybir
import concourse.tile as tile
from concourse import bacc
from concourse.bass_utils import run_bass_kernel_spmd

DT = mybir.dt
AF = mybir.ActivationFunctionType
OP = mybir.AluOpType

B, S, D, H, HD = 2, 2048, 1024, 16, 64
NCORES = 8
HPC = 4            # heads per core
ESL = HPC * HD     # 256
QH = 2             # q halves
QW = S // QH       # 1024
NKT = S // 128     # 16 k-tiles
NDC = D // 128     # 8 d-chunks

_cache = {}


def _build():
    if "nc" in _cache:
        return _cache["nc"]
    nc = bacc.Bacc(None, target_bir_lowering=False)

    xT_d = nc.dram_tensor("xT", [D, S], DT.float16, kind="ExternalInput")
    wq_d = nc.dram_tensor("wq", [D, ESL], DT.float16, kind="ExternalInput")
    wk_d = nc.dram_tensor("wk", [D, ESL], DT.float16, kind="ExternalInput")
    wv_d = nc.dram_tensor("wv", [D, ESL], DT.float16, kind="ExternalInput")
    wo_d = nc.dram_tensor("wo", [ESL, D], DT.float16, kind="ExternalInput")
    bq_d = nc.dram_tensor("bq", [128, 2], DT.float32, kind="ExternalInput")
    bk_d = nc.dram_tensor("bk", [128, 2], DT.float32, kind="ExternalInput")
    mb_d = nc.dram_tensor("mb", [128, NKT], DT.float32, kind="ExternalInput")
    accT_d = nc.dram_tensor("accT", [S, S], DT.float16, kind="ExternalOutput")
    outT_d = nc.dram_tensor("outT", [D, S], DT.float32, kind="ExternalOutput")

    with tile.TileContext(nc) as tc:
        with tc.tile_pool(name="persist", bufs=1) as persist:
            qTs = persist.tile([128, 2, S], DT.float16)
            kTs = persist.tile([128, 2, S], DT.float16)
            vv = persist.tile([128, HPC, NKT, 68], DT.float16)
            wos = persist.tile([128, 2, D], DT.float16)
            ctxT2 = persist.tile([128, 2, S], DT.float16)
            bq_t = persist.tile([128, 2], DT.float32)
            bk_t = persist.tile([128, 2], DT.float32)
            mb_t = persist.tile([128, NKT], DT.float32)

            nc.sync.dma_start(out=wos[:], in_=wo_d[:].rearrange("(c p) f -> p c f", p=128))
            nc.sync.dma_start(out=bq_t[:], in_=bq_d[:])
            nc.sync.dma_start(out=bk_t[:], in_=bk_d[:])
            nc.sync.dma_start(out=mb_t[:], in_=mb_d[:])
            nc.vector.memset(vv[:], 1.0)

            # ---------------- Phase P: projections ----------------
            with tc.tile_pool(name="px", bufs=1) as px, \
                 tc.tile_pool(name="ppqk", bufs=4, space="PSUM") as ppqk, \
                 tc.tile_pool(name="ppv", bufs=2, space="PSUM") as ppv:
                xTs = px.tile([128, NDC, S], DT.float16)
                wqs = px.tile([128, NDC, ESL], DT.float16)
                wks = px.tile([128, NDC, ESL], DT.float16)
                wvs = px.tile([128, NDC, ESL], DT.float16)
                xT_r = xT_d[:].rearrange("(c p) s -> p c s", p=128)
                for dc in range(NDC):
                    nc.sync.dma_start(out=xTs[:, dc, :], in_=xT_r[:, dc, :])
                nc.sync.dma_start(out=wqs[:], in_=wq_d[:].rearrange("(c p) e -> p c e", p=128))
                nc.sync.dma_start(out=wks[:], in_=wk_d[:].rearrange("(c p) e -> p c e", p=128))
                nc.sync.dma_start(out=wvs[:], in_=wv_d[:].rearrange("(c p) e -> p c e", p=128))

                # q/k projections -> transposed [e, s] layout (1024-wide moving)
                for wsrc, bias_t, dst in ((wks, bk_t, kTs), (wqs, bq_t, qTs)):
                    for ec in range(2):
                        pss = [ppqk.tile([128, 512], DT.float32, tag="ppqk",
                                         name=f"psqk{sb}") for sb in range(4)]
                        for dc in range(NDC):
                            for sb in range(4):
                                nc.tensor.matmul(
                                    pss[sb][:],
                                    lhsT=wsrc[:, dc, ec * 128:(ec + 1) * 128],
                                    rhs=xTs[:, dc, sb * 512:(sb + 1) * 512],
                                    start=(dc == 0), stop=(dc == NDC - 1),
                                )
                        for sb in range(4):
                            nc.vector.tensor_scalar(
                                out=dst[:, ec, sb * 512:(sb + 1) * 512],
                                in0=pss[sb][:], scalar1=bias_t[:, ec:ec + 1],
                                scalar2=None, op0=OP.add,
                            )

                # v projection -> natural [s, e] layout, per-head slices of vv
                for sc in range(NKT):
                    ps = ppv.tile([128, ESL], DT.float32, tag="ppv")
                    for dc in range(NDC):
                        nc.tensor.matmul(
                            ps[:],
                            lhsT=xTs[:, dc, sc * 128:(sc + 1) * 128],
                            rhs=wvs[:, dc, :],
                            start=(dc == 0), stop=(dc == NDC - 1),
                        )
                    for h in range(HPC):
                        nc.scalar.activation(
                            vv[:, h, sc, 0:HD], ps[:, h * HD:(h + 1) * HD], AF.Copy,
                        )

            # ---------------- Phase A: attention ----------------
            with tc.tile_pool(name="pa_exp", bufs=2) as pa_exp, \
                 tc.tile_pool(name="pa_sm", bufs=2) as pa_sm, \
                 tc.tile_pool(name="pa_z", bufs=2) as pa_z, \
                 tc.tile_pool(name="pa_zb", bufs=2) as pa_zb, \
                 tc.tile_pool(name="pa_cu", bufs=2) as pa_cu, \
                 tc.tile_pool(name="pa_acc", bufs=2) as pa_acc, \
                 tc.tile_pool(name="ps_sc", bufs=2, space="PSUM") as ps_sc, \
                 tc.tile_pool(name="ps_ctx", bufs=1, space="PSUM") as ps_ctx:
                for qh in range(QH):
                    qsl = slice(qh * QW, (qh + 1) * QW)
                    acc = pa_acc.tile([128, NKT, QW], DT.float16, tag="acc")
                    for h in range(HPC):
                        hc, hp = h // 2, (h % 2) * 64
                        expT_h = pa_exp.tile([128, NKT, QW], DT.float16, tag="expT")
                        ctxp = ps_ctx.tile([65, QW], DT.float32, tag="ctxp")

                        def sc_step(kt):
                            scp = ps_sc.tile([128, QW], DT.float32, tag="scp",
                                             name=f"scp{kt % 2}")
                            for qq in range(2):
                                nc.tensor.matmul(
                                    scp[:, qq * 512:(qq + 1) * 512],
                                    lhsT=kTs[hp:hp + 64, hc, kt * 128:(kt + 1) * 128],
                                    rhs=qTs[hp:hp + 64, hc,
                                            qh * QW + qq * 512: qh * QW + (qq + 1) * 512],
                                    start=True, stop=True,
                                )
                            nc.scalar.activation(
                                expT_h[:, kt, :], scp[:], AF.Exp,
                                bias=mb_t[:, kt:kt + 1],
                            )

                        def av_step(kt):
                            for qq in range(2):
                                nc.tensor.matmul(
                                    ctxp[:, qq * 512:(qq + 1) * 512],
                                    lhsT=vv[:, h, kt, 0:65],
                                    rhs=expT_h[:, kt, qq * 512:(qq + 1) * 512],
                                    start=(kt == 0), stop=(kt == NKT - 1),
                                )

                        # sc runs ahead; avs issued in 4-kt accumulation bursts
                        GB = 4
                        for g in range(NKT // GB):
                            for kt in range(g * GB, (g + 1) * GB):
                                sc_step(kt)
                            if g > 0:
                                for kt in range((g - 1) * GB, g * GB):
                                    av_step(kt)
                        for kt in range(NKT - GB, NKT):
                            av_step(kt)

                        # per-head Z chain: evac -> compact -> recip -> broadcast
                        zal = pa_z.tile([1, QW], DT.float16, tag="zal")
                        nc.vector.tensor_copy(zal[0:1, :], ctxp[64:65, :])
                        ctxu = pa_cu.tile([65, QW], DT.float16, tag="ctxu")
                        nc.vector.tensor_copy(ctxu[:], ctxp[:])
                        zc = pa_z.tile([128, QW // 128], DT.float16, tag="zc")
                        nc.gpsimd.dma_start(out=zc[:], in_=zal[0:1, :])
                        zcf = pa_z.tile([128, QW // 128], DT.float32, tag="zcf")
                        nc.vector.tensor_copy(zcf[:], zc[:])
                        zci = pa_z.tile([128, QW // 128], DT.float32, tag="zci")
                        nc.vector.reciprocal(zci[:], zcf[:])
                        zch = pa_z.tile([128, QW // 128], DT.float16, tag="zch")
                        nc.vector.tensor_copy(zch[:], zci[:])
                        zrow = pa_z.tile([1, QW], DT.float16, tag="zrow")
                        nc.gpsimd.dma_start(out=zrow[0:1, :], in_=zch[:])
                        zbb = pa_zb.tile([128, QW], DT.float16, tag="zbb")
                        nc.gpsimd.partition_broadcast(zbb[:], zrow[0:1, :])

                        # normalized ctxT slice (fp16) for the out-projection
                        nc.vector.tensor_tensor(
                            out=ctxT2[hp:hp + 64, hc, qsl],
                            in0=ctxu[0:64, :], in1=zbb[0:64, :], op=OP.mult,
                        )
                        # attn-mean partial: acc[kt] (+)= expT_h[kt] * (1/Z_h)
                        for kt in range(NKT):
                            if h == 0:
                                nc.vector.tensor_tensor(
                                    out=acc[:, kt, :], in0=expT_h[:, kt, :],
                                    in1=zbb[:], op=OP.mult,
                                )
                            else:
                                tmp = pa_sm.tile([128, QW], DT.float16, tag="tmp")
                                nc.vector.tensor_tensor(
                                    out=tmp[:], in0=expT_h[:, kt, :], in1=zbb[:],
                                    op=OP.mult,
                                )
                                nc.vector.tensor_tensor(
                                    out=acc[:, kt, :], in0=acc[:, kt, :],
                                    in1=tmp[:], op=OP.add,
                                )
                    for kt in range(NKT):
                        nc.sync.dma_start(
                            out=accT_d[kt * 128:(kt + 1) * 128, qsl],
                            in_=acc[:, kt, :],
                        )

                    # out projection for this s-half (dense accumulation burst)
                    for ft in range(D // 128):
                        pos = [ps_ctx.tile([128, 512], DT.float32, tag="pso",
                                           name=f"pso{i}") for i in range(2)]
                        for ec in range(2):
                            for sb in range(2):
                                nc.tensor.matmul(
                                    pos[sb][:],
                                    lhsT=wos[:, ec, ft * 128:(ft + 1) * 128],
                                    rhs=ctxT2[:, ec, qh * QW + sb * 512:
                                              qh * QW + (sb + 1) * 512],
                                    start=(ec == 0), stop=(ec == 1),
                                )
                        for sb in range(2):
                            ot = pa_sm.tile([128, 512], DT.float32, tag="ot")
                            nc.scalar.activation(ot[:], pos[sb][:], AF.Copy)
                            nc.sync.dma_start(
                                out=outT_d[ft * 128:(ft + 1) * 128,
                                           qh * QW + sb * 512: qh * QW + (sb + 1) * 512],
                                in_=ot[:],
                            )

    nc.compile()
    _cache["nc"] = nc
    return nc


def _prep_inputs(x, mask, Wq, bq, Wk, bk, Wv, bv, Wo, bo):
    """Build the 8 per-core input maps (host-side shard + transpose)."""
    x = np.asarray(x, np.float32)
    mask = np.asarray(mask)
    Wq = np.asarray(Wq, np.float32); bq = np.asarray(bq, np.float32)
    Wk = np.asarray(Wk, np.float32); bk = np.asarray(bk, np.float32)
    Wv = np.asarray(Wv, np.float32)
    Wo = np.asarray(Wo, np.float32)

    WqT = (Wq.T / 8.0).astype(np.float16)   # scores scale folded in
    WkT = Wk.T.astype(np.float16)
    WvT = Wv.T.astype(np.float16)
    WoT = Wo.T.astype(np.float16)

    in_maps = []
    for c in range(NCORES):
        b, hg = c // HPC, c % HPC
        esl = slice(hg * ESL, (hg + 1) * ESL)
        mb = (-1e9 * (1.0 - mask[b].astype(np.float32)))
        in_maps.append({
            "xT": np.ascontiguousarray(x[b].T.astype(np.float16)),
            "wq": np.ascontiguousarray(WqT[:, esl]),
            "wk": np.ascontiguousarray(WkT[:, esl]),
            "wv": np.ascontiguousarray(WvT[:, esl]),
            "wo": np.ascontiguousarray(WoT[esl, :]),
            "bq": np.ascontiguousarray((bq[esl] / 8.0).reshape(2, 128).T),
            "bk": np.ascontiguousarray(bk[esl].reshape(2, 128).T),
            "mb": np.ascontiguousarray(mb.reshape(NKT, 128).T),
        })
    return in_maps


def _run(inputs, trace=False):
    nc = _build()
    in_maps = _prep_inputs(**{k: inputs[k] for k in
                              ("x", "mask", "Wq", "bq", "Wk", "bk",
                               "Wv", "bv", "Wo", "bo")})
    res = run_bass_kernel_spmd(nc, in_maps, core_ids=list(range(NCORES)),
                               trace=trace)
    bv = np.asarray(inputs["bv"], np.float32)
    bo = np.asarray(inputs["bo"], np.float32)
    Wo = np.asarray(inputs["Wo"], np.float32)
    corr = bv @ Wo.T + bo   # sum_k p_k = 1 makes bv a constant additive term

    out = np.empty((B, S, D), np.float32)
    attn = np.empty((B, S, S), np.float32)
    for b in range(B):
        outT = np.zeros((D, S), np.float32)
        accT = np.zeros((S, S), np.float32)
        for hg in range(HPC):
            r = res.results[b * HPC + hg]
            outT += r["outT"]
            accT += r["accT"].astype(np.float32)
        out[b] = outT.T + corr
        attn[b] = accT.T / float(H)
    return (out, attn), res


def kernel(**inputs):
    (out, attn), _ = _run(inputs, trace=False)
    return out, attn


# revision 23
# speedup vs baseline: 1.2160x; 1.2160x over previous
"""Bass/Tile TRN2 kernel for nn_MultiHeadSelfAttention (B=2, S=2048, D=1024, H=16).

Sharding: 8 cores; core c handles batch b=c//4 and the 4 heads hg=c%4
(e-slice of 256 columns of the fused QKV/out projections).

Per-core device program (SPMD — same NEFF on every core, different data):
  - fp16 everywhere on the PE (1 cycle/row, 1024-wide moving operands,
    fast weight loads); PSUM accumulation is fp32.
  - Q/K projections into transposed layout qT/kT [e, s]; V projection into
    natural layout [s, e] with an appended ones column (gives Z for free).
  - Per (q-half, head): scoresT[k, q] = kT_h^T @ qT_h (d=64 contraction),
    exp on ACT (key-padding mask folded in as a per-partition bias),
    ctx~T[65, q] = vv_h^T @ expT accumulated over k in PSUM (row 64 = Z).
    The PE stream is software-pipelined: scores run ahead and the attnV
    matmuls are issued in 4-k-tile accumulation bursts so the PE rarely
    stalls on ACT and the HAM clock stays mostly at 2.4 GHz.
  - Per-head Z chain off the critical path: the Z row is compacted to a
    [128, 8] layout via a small DMA (full-lane reciprocal), then 1/Z is
    partition-broadcast on the (otherwise idle) GpSimd engine.
  - attn-mean partial (kt-major): acc[k,q] = sum_h expT_h * (1/Z_h), DVE fp16.
  - out-proj per q-half: outT[f, s] partial = Wo-slice chunks^T @ ctxT,
    emitted right after each half's attention as a dense warm burst.
Host: sums the 4 per-batch core partials, transposes, adds bias terms.
"""

import sys

sys.path.insert(0, "/opt/trn_rl_repo")

import numpy as np
import concourse.bass as bass  # noqa: F401
import concourse.mybir as mybir
import concourse.tile as tile
from concourse import bacc
from concourse.bass_utils import run_bass_kernel_spmd

DT = mybir.dt
AF = mybir.ActivationFunctionType
OP = mybir.AluOpType

B, S, D, H, HD = 2, 2048, 1024, 16, 64
NCORES = 8
HPC = 4            # heads per core
ESL = HPC * HD     # 256
QH = 2             # q halves
QW = S // QH       # 1024
NKT = S // 128     # 16 k-tiles
NDC = D // 128     # 8 d-chunks

_cache = {}


def _build():
    if "nc" in _cache:
        return _cache["nc"]
    nc = bacc.Bacc(None, target_bir_lowering=False)

    xT_d = nc.dram_tensor("xT", [D, S], DT.float16, kind="ExternalInput")
    wq_d = nc.dram_tensor("wq", [D, ESL], DT.float16, kind="ExternalInput")
    wk_d = nc.dram_tensor("wk", [D, ESL], DT.float16, kind="ExternalInput")
    wv_d = nc.dram_tensor("wv", [D, ESL], DT.float16, kind="ExternalInput")
    wo_d = nc.dram_tensor("wo", [ESL, D], DT.float16, kind="ExternalInput")
    bq_d = nc.dram_tensor("bq", [128, 2], DT.float32, kind="ExternalInput")
    bk_d = nc.dram_tensor("bk", [128, 2], DT.float32, kind="ExternalInput")
    mb_d = nc.dram_tensor("mb", [128, NKT], DT.float32, kind="ExternalInput")
    accT_d = nc.dram_tensor("accT", [S, S], DT.float16, kind="ExternalOutput")
    outT_d = nc.dram_tensor("outT", [D, S], DT.float32, kind="ExternalOutput")

    with tile.TileContext(nc) as tc:
        with tc.tile_pool(name="persist", bufs=1) as persist:
            qTs = persist.tile([128, 2, S], DT.float16)
            kTs = persist.tile([128, 2, S], DT.float16)
            vv = persist.tile([128, HPC, NKT, 68], DT.float16)
            wos = persist.tile([128, 2, D], DT.float16)
            ctxT2 = persist.tile([128, 2, S], DT.float16)
            bq_t = persist.tile([128, 2], DT.float32)
            bk_t = persist.tile([128, 2], DT.float32)
            mb_t = persist.tile([128, NKT], DT.float32)

            nc.sync.dma_start(out=wos[:], in_=wo_d[:].rearrange("(c p) f -> p c f", p=128))
            nc.sync.dma_start(out=bq_t[:], in_=bq_d[:])
            nc.sync.dma_start(out=bk_t[:], in_=bk_d[:])
            nc.sync.dma_start(out=mb_t[:], in_=mb_d[:])
            nc.vector.memset(vv[:], 1.0)

            # ---------------- Phase P: projections ----------------
            with tc.tile_pool(name="px", bufs=1) as px, \
                 tc.tile_pool(name="ppqk", bufs=4, space="PSUM") as ppqk, \
                 tc.tile_pool(name="ppv", bufs=2, space="PSUM") as ppv:
                xTs = px.tile([128, NDC, S], DT.float16)
                wqs = px.tile([128, NDC, ESL], DT.float16)
                wks = px.tile([128, NDC, ESL], DT.float16)
                wvs = px.tile([128, NDC, ESL], DT.float16)
                xT_r = xT_d[:].rearrange("(c p) s -> p c s", p=128)
                for dc in range(NDC):
                    nc.sync.dma_start(out=xTs[:, dc, :], in_=xT_r[:, dc, :])
                nc.sync.dma_start(out=wqs[:], in_=wq_d[:].rearrange("(c p) e -> p c e", p=128))
                nc.sync.dma_start(out=wks[:], in_=wk_d[:].rearrange("(c p) e -> p c e", p=128))
                nc.sync.dma_start(out=wvs[:], in_=wv_d[:].rearrange("(c p) e -> p c e", p=128))

                # q/k projections -> transposed [e, s] layout (1024-wide moving)
                for wsrc, bias_t, dst in ((wks, bk_t, kTs), (wqs, bq_t, qTs)):
                    for ec in range(2):
                        pss = [ppqk.tile([128, 512], DT.float32, tag="ppqk",
                                         name=f"psqk{sb}") for sb in range(4)]
                        for dc in range(NDC):
                            for sb in range(4):
                                nc.tensor.matmul(
                                    pss[sb][:],
                                    lhsT=wsrc[:, dc, ec * 128:(ec + 1) * 128],
                                    rhs=xTs[:, dc, sb * 512:(sb + 1) * 512],
                                    start=(dc == 0), stop=(dc == NDC - 1),
                                )
                        for sb in range(4):
                            nc.vector.tensor_scalar(
                                out=dst[:, ec, sb * 512:(sb + 1) * 512],
                                in0=pss[sb][:], scalar1=bias_t[:, ec:ec + 1],
                                scalar2=None, op0=OP.add,
                            )

                # v projection -> natural [s, e] layout, per-head slices of vv
                for sc in range(NKT):
                    ps = ppv.tile([128, ESL], DT.float32, tag="ppv")
                    for dc in range(NDC):
                        nc.tensor.matmul(
                            ps[:],
                            lhsT=xTs[:, dc, sc * 128:(sc + 1) * 128],
                            rhs=wvs[:, dc, :],
                            start=(dc == 0), stop=(dc == NDC - 1),
                        )
                    for h in range(HPC):
                        nc.scalar.activation(
                            vv[:, h, sc, 0:HD], ps[:, h * HD:(h + 1) * HD], AF.Copy,
                        )

            # ---------------- Phase A: attention ----------------
            with tc.tile_pool(name="pa_exp", bufs=2) as pa_exp, \
                 tc.tile_pool(name="pa_sm", bufs=2) as pa_sm, \
                 tc.tile_pool(name="pa_z", bufs=2) as pa_z, \
                 tc.tile_pool(name="pa_zb", bufs=2) as pa_zb, \
                 tc.tile_pool(name="pa_cu", bufs=2) as pa_cu, \
                 tc.tile_pool(name="pa_acc", bufs=2) as pa_acc, \
                 tc.tile_pool(name="ps_sc", bufs=2, space="PSUM") as ps_sc, \
                 tc.tile_pool(name="ps_ctx", bufs=1, space="PSUM") as ps_ctx:
                for qh in range(QH):
                    qsl = slice(qh * QW, (qh + 1) * QW)
                    acc = pa_acc.tile([128, NKT, QW], DT.float16, tag="acc")
                    for h in range(HPC):
                        hc, hp = h // 2, (h % 2) * 64
                        expT_h = pa_exp.tile([128, NKT, QW], DT.float16, tag="expT")
                        ctxp = ps_ctx.tile([65, QW], DT.float32, tag="ctxp")

                        def sc_step(kt):
                            scp = ps_sc.tile([128, QW], DT.float32, tag="scp",
                                             name=f"scp{kt % 2}")
                            for qq in range(2):
                                nc.tensor.matmul(
                                    scp[:, qq * 512:(qq + 1) * 512],
                                    lhsT=kTs[hp:hp + 64, hc, kt * 128:(kt + 1) * 128],
                                    rhs=qTs[hp:hp + 64, hc,
                                            qh * QW + qq * 512: qh * QW + (qq + 1) * 512],
                                    start=True, stop=True,
                                )
                            nc.scalar.activation(
                                expT_h[:, kt, :], scp[:], AF.Exp,
                                bias=mb_t[:, kt:kt + 1],
                            )

                        def av_step(kt):
                            for qq in range(2):
                                nc.tensor.matmul(
                                    ctxp[:, qq * 512:(qq + 1) * 512],
                                    lhsT=vv[:, h, kt, 0:65],
                                    rhs=expT_h[:, kt, qq * 512:(qq + 1) * 512],
                                    start=(kt == 0), stop=(kt == NKT - 1),
                                )

                        # sc runs ahead; avs issued in 4-kt accumulation bursts
                        GB = 4
                        for g in range(NKT // GB):
                            for kt in range(g * GB, (g + 1) * GB):
                                sc_step(kt)
                            if g > 0:
                                for kt in range((g - 1) * GB, g * GB):
                                    av_step(kt)
                        for kt in range(NKT - GB, NKT):
                            av_step(kt)

                        # per-head Z chain: evac -> compact -> recip -> broadcast
                        zal = pa_z.tile([1, QW], DT.float16, tag="zal")
                        nc.vector.tensor_copy(zal[0:1, :], ctxp[64:65, :])
                        ctxu = pa_cu.tile([65, QW], DT.float16, tag="ctxu")
                        nc.vector.tensor_copy(ctxu[:], ctxp[:])
                        zc = pa_z.tile([128, QW // 128], DT.float16, tag="zc")
                        nc.gpsimd.dma_start(out=zc[:], in_=zal[0:1, :])
                        zcf = pa_z.tile([128, QW // 128], DT.float32, tag="zcf")
                        nc.vector.tensor_copy(zcf[:], zc[:])
                        zci = pa_z.tile([128, QW // 128], DT.float32, tag="zci")
                        nc.vector.reciprocal(zci[:], zcf[:])
                        zch = pa_z.tile([128, QW // 128], DT.float16, tag="zch")
                        nc.vector.tensor_copy(zch[:], zci[:])
                        zrow = pa_z.tile([1, QW], DT.float16, tag="zrow")
                        nc.gpsimd.dma_start(out=zrow[0:1, :], in_=zch[:])
                        zbb = pa_zb.tile([128, QW], DT.float16, tag="zbb")
                        nc.gpsimd.partition_broadcast(zbb[:], zrow[0:1, :])

                        # normalized ctxT slice (fp16) for the out-projection
                        nc.vector.tensor_tensor(
                            out=ctxT2[hp:hp + 64, hc, qsl],
                            in0=ctxu[0:64, :], in1=zbb[0:64, :], op=OP.mult,
                        )
                        # attn-mean partial: acc[kt] (+)= expT_h[kt] * (1/Z_h)
                        for kt in range(NKT):
                            if h == 0:
                                nc.vector.tensor_tensor(
                                    out=acc[:, kt, :], in0=expT_h[:, kt, :],
                                    in1=zbb[:], op=OP.mult,
                                )
                            else:
                                tmp = pa_sm.tile([128, QW], DT.float16, tag="tmp")
                                nc.vector.tensor_tensor(
                                    out=tmp[:], in0=expT_h[:, kt, :], in1=zbb[:],
                                    op=OP.mult,
                                )
                                nc.vector.tensor_tensor(
                                    out=acc[:, kt, :], in0=acc[:, kt, :],
                                    in1=tmp[:], op=OP.add,
                                )
                    for kt in range(NKT):
                        nc.sync.dma_start(
                            out=accT_d[kt * 128:(kt + 1) * 128, qsl],
                            in_=acc[:, kt, :],
                        )

                    # out projection for this s-half (dense accumulation burst)
                    for ft in range(D // 128):
                        pos = [ps_ctx.tile([128, 512], DT.float32, tag="pso",
                                           name=f"pso{i}") for i in range(2)]
                        for ec in range(2):
                            for sb in range(2):
                                nc.tensor.matmul(
                                    pos[sb][:],
                                    lhsT=wos[:, ec, ft * 128:(ft + 1) * 128],
                                    rhs=ctxT2[:, ec, qh * QW + sb * 512:
                                              qh * QW + (sb + 1) * 512],
                                    start=(ec == 0), stop=(ec == 1),
                                )
                        for sb in range(2):
                            ot = pa_sm.tile([128, 512], DT.float32, tag="ot")
                            nc.scalar.activation(ot[:], pos[sb][:], AF.Copy)
                            nc.sync.dma_start(
                                out=outT_d[ft * 128:(ft + 1) * 128,
                                           qh * QW + sb * 512: qh * QW + (sb + 1) * 512],
                                in_=ot[:],
                            )

    nc.compile()
    _cache["nc"] = nc
    return nc


def _prep_inputs(x, mask, Wq, bq, Wk, bk, Wv, bv, Wo, bo):
    """Build the 8 per-core input maps (host-side shard + transpose)."""
    x = np.asarray(x, np.float32)
    mask = np.asarray(mask)
    Wq = np.asarray(Wq, np.float32); bq = np.asarray(bq, np.float32)
    Wk = np.asarray(Wk, np.float32); bk = np.asarray(bk, np.float32)
    Wv = np.asarray(Wv, np.float32)
    Wo = np.asarray(Wo, np.float32)

    WqT = (Wq.T / 8.0).astype(np.float16)   # scores scale folded in
    WkT = Wk.T.astype(np.float16)
    WvT = Wv.T.astype(np.float16)
    WoT = Wo.T.astype(np.float16)

    in_maps = []
    for c in range(NCORES):
        b, hg = c // HPC, c % HPC
        esl = slice(hg * ESL, (hg + 1) * ESL)
        mb = (-1e9 * (1.0 - mask[b].astype(np.float32)))
        in_maps.append({
            "xT": np.ascontiguousarray(x[b].T.astype(np.float16)),
            "wq": np.ascontiguousarray(WqT[:, esl]),
            "wk": np.ascontiguousarray(WkT[:, esl]),
            "wv": np.ascontiguousarray(WvT[:, esl]),
            "wo": np.ascontiguousarray(WoT[esl, :]),
            "bq": np.ascontiguousarray((bq[esl] / 8.0).reshape(2, 128).T),
            "bk": np.ascontiguousarray(bk[esl].reshape(2, 128).T),
            "mb": np.ascontiguousarray(mb.reshape(NKT, 128).T),
        })
    return in_maps


def _run(inputs, trace=False):
    nc = _build()
    in_maps = _prep_inputs(**{k: inputs[k] for k in
                              ("x", "mask", "Wq", "bq", "Wk", "bk",
                               "Wv", "bv", "Wo", "bo")})
    res = run_bass_kernel_spmd(nc, in_maps, core_ids=list(range(NCORES)),
                               trace=trace)
    bv = np.asarray(inputs["bv"], np.float32)
    bo = np.asarray(inputs["bo"], np.float32)
    Wo = np.asarray(inputs["Wo"], np.float32)
    corr = bv @ Wo.T + bo   # sum_k p_k = 1 makes bv a constant additive term

    out = np.empty((B, S, D), np.float32)
    attn = np.empty((B, S, S), np.float32)
    for b in range(B):
        outT = np.zeros((D, S), np.float32)
        accT = np.zeros((S, S), np.float32)
        for hg in range(HPC):
            r = res.results[b * HPC + hg]
            outT += r["outT"]
            accT += r["accT"].astype(np.float32)
        out[b] = outT.T + corr
        attn[b] = accT.T / float(H)
    return (out, attn), res


def kernel(**inputs):
    (out, attn), _ = _run(inputs, trace=False)
    return out, attn


# revision 30
# speedup vs baseline: 1.2471x; 1.0256x over previous
"""Bass/Tile TRN2 kernel for nn_MultiHeadSelfAttention (B=2, S=2048, D=1024, H=16).

Sharding: 8 cores; core c handles batch b=c//4 and the 4 heads hg=c%4
(e-slice of 256 columns of the fused QKV/out projections).

Per-core device program (SPMD — same NEFF on every core, different data):
  - fp16 everywhere on the PE (1 cycle/row, 1024-wide moving operands,
    fast weight loads); PSUM accumulation is fp32.
  - Q/K projections into transposed layout qT/kT [e, s]; V projection into
    natural layout [s, e] with an appended ones column (gives Z for free).
  - Per (q-half, head): scoresT[k, q] = kT_h^T @ qT_h (d=64 contraction),
    exp on ACT (key-padding mask folded in as a per-partition bias),
    ctx~T[65, q] = vv_h^T @ expT accumulated over k in PSUM (row 64 = Z).
    The PE stream is software-pipelined: scores run ahead and the attnV
    matmuls are issued in 4-k-tile accumulation bursts so the PE rarely
    stalls on ACT and the HAM clock stays mostly at 2.4 GHz.
  - Per-head Z chain off the critical path: the Z row is compacted to a
    [128, 8] layout via a small DMA (full-lane reciprocal), then 1/Z is
    partition-broadcast on the (otherwise idle) GpSimd engine.
  - attn-mean partial (kt-major): acc[k,q] = sum_h expT_h * (1/Z_h), DVE fp16.
  - out-proj per q-half: outT[f, s] partial = Wo-slice chunks^T @ ctxT,
    emitted right after each half's attention as a dense warm burst.
Host: sums the 4 per-batch core partials, transposes, adds bias terms.
"""

import sys

sys.path.insert(0, "/opt/trn_rl_repo")

import numpy as np
import concourse.bass as bass  # noqa: F401
import concourse.mybir as mybir
import concourse.tile as tile
from concourse import bacc
from concourse.bass_utils import run_bass_kernel_spmd

DT = mybir.dt
AF = mybir.ActivationFunctionType
OP = mybir.AluOpType

B, S, D, H, HD = 2, 2048, 1024, 16, 64
NCORES = 8
HPC = 4            # heads per core
ESL = HPC * HD     # 256
QH = 2             # q halves
QW = S // QH       # 1024
NKT = S // 128     # 16 k-tiles
NDC = D // 128     # 8 d-chunks

_cache = {}


def _build():
    if "nc" in _cache:
        return _cache["nc"]
    nc = bacc.Bacc(None, target_bir_lowering=False)

    xT_d = nc.dram_tensor("xT", [D, S], DT.float16, kind="ExternalInput")
    wq_d = nc.dram_tensor("wq", [D, ESL], DT.float16, kind="ExternalInput")
    wk_d = nc.dram_tensor("wk", [D, ESL], DT.float16, kind="ExternalInput")
    wv_d = nc.dram_tensor("wv", [D, ESL], DT.float16, kind="ExternalInput")
    wo_d = nc.dram_tensor("wo", [ESL, D], DT.float16, kind="ExternalInput")
    bq_d = nc.dram_tensor("bq", [128, 2], DT.float32, kind="ExternalInput")
    bk_d = nc.dram_tensor("bk", [128, 2], DT.float32, kind="ExternalInput")
    mb_d = nc.dram_tensor("mb", [128, NKT], DT.float32, kind="ExternalInput")
    accT_d = nc.dram_tensor("accT", [S, S], DT.float16, kind="ExternalOutput")
    outT_d = nc.dram_tensor("outT", [D, S], DT.float32, kind="ExternalOutput")

    with tile.TileContext(nc) as tc:
        with tc.tile_pool(name="persist", bufs=1) as persist:
            qTs = persist.tile([128, 2, S], DT.float16)
            kTs = persist.tile([128, 2, S], DT.float16)
            vv = persist.tile([128, HPC, NKT, 68], DT.float16)
            wos = persist.tile([128, 2, D], DT.float16)
            ctxT2 = persist.tile([128, 2, S], DT.float16)
            bq_t = persist.tile([128, 2], DT.float32)
            bk_t = persist.tile([128, 2], DT.float32)
            mb_t = persist.tile([128, NKT], DT.float32)

            nc.sync.dma_start(out=wos[:], in_=wo_d[:].rearrange("(c p) f -> p c f", p=128))
            nc.sync.dma_start(out=bq_t[:], in_=bq_d[:])
            nc.sync.dma_start(out=bk_t[:], in_=bk_d[:])
            nc.sync.dma_start(out=mb_t[:], in_=mb_d[:])
            nc.vector.memset(vv[:], 1.0)

            # ---------- projections + attention, overlapped ----------
            with tc.tile_pool(name="pa_exp", bufs=2) as pa_exp, \
                 tc.tile_pool(name="pa_sm", bufs=2) as pa_sm, \
                 tc.tile_pool(name="pa_z", bufs=2) as pa_z, \
                 tc.tile_pool(name="pa_zb", bufs=2) as pa_zb, \
                 tc.tile_pool(name="pa_cu", bufs=2) as pa_cu, \
                 tc.tile_pool(name="pa_acc", bufs=1) as pa_acc, \
                 tc.tile_pool(name="ps_sc", bufs=2, space="PSUM") as ps_sc:

                def make_sc_step(expT_h, h, qh):
                    hc, hp = h // 2, (h % 2) * 64

                    def sc_step(kt):
                        scp = ps_sc.tile([128, QW], DT.float32, tag="scp",
                                         name=f"scp{kt % 2}")
                        for qq in range(2):
                            nc.tensor.matmul(
                                scp[:, qq * 512:(qq + 1) * 512],
                                lhsT=kTs[hp:hp + 64, hc, kt * 128:(kt + 1) * 128],
                                rhs=qTs[hp:hp + 64, hc,
                                        qh * QW + qq * 512: qh * QW + (qq + 1) * 512],
                                start=True, stop=True,
                            )
                        nc.scalar.activation(
                            expT_h[:, kt, :], scp[:], AF.Exp,
                            bias=mb_t[:, kt:kt + 1],
                        )
                    return sc_step

                # ----- phase P pools: open only until proj thunks drain -----
                expT0 = pa_exp.tile([128, NKT, QW], DT.float16, tag="expT",
                                    name="expT0")
                with tc.tile_pool(name="px", bufs=1) as px, \
                     tc.tile_pool(name="ppqk", bufs=2, space="PSUM") as ppqk, \
                     tc.tile_pool(name="ppv", bufs=2, space="PSUM") as ppv:
                    xTs = px.tile([128, NDC, S], DT.float16)
                    wqs = px.tile([128, NDC, ESL], DT.float16)
                    wks = px.tile([128, NDC, ESL], DT.float16)
                    wvs = px.tile([128, NDC, ESL], DT.float16)
                    nc.sync.dma_start(out=wks[:], in_=wk_d[:].rearrange("(c p) e -> p c e", p=128))
                    nc.sync.dma_start(out=wqs[:], in_=wq_d[:].rearrange("(c p) e -> p c e", p=128))
                    xT_r = xT_d[:].rearrange("(c p) s -> p c s", p=128)
                    for dc in range(NDC):
                        nc.sync.dma_start(out=xTs[:, dc, :], in_=xT_r[:, dc, :])
                    nc.sync.dma_start(out=wvs[:], in_=wv_d[:].rearrange("(c p) e -> p c e", p=128))

                    def qk_proj(wsrc, bias_t, dst, ec, sb):
                        ps = ppqk.tile([128, 512], DT.float32, tag="ppqk")
                        for dc in range(NDC):
                            nc.tensor.matmul(
                                ps[:],
                                lhsT=wsrc[:, dc, ec * 128:(ec + 1) * 128],
                                rhs=xTs[:, dc, sb * 512:(sb + 1) * 512],
                                start=(dc == 0), stop=(dc == NDC - 1),
                            )
                        nc.vector.tensor_scalar(
                            out=dst[:, ec, sb * 512:(sb + 1) * 512],
                            in0=ps[:], scalar1=bias_t[:, ec:ec + 1],
                            scalar2=None, op0=OP.add,
                        )

                    def v_proj(sc):
                        ps = ppv.tile([128, ESL], DT.float32, tag="ppv")
                        for dc in range(NDC):
                            nc.tensor.matmul(
                                ps[:],
                                lhsT=xTs[:, dc, sc * 128:(sc + 1) * 128],
                                rhs=wvs[:, dc, :],
                                start=(dc == 0), stop=(dc == NDC - 1),
                            )
                        for h in range(HPC):
                            nc.scalar.activation(
                                vv[:, h, sc, 0:HD], ps[:, h * HD:(h + 1) * HD],
                                AF.Copy,
                            )

                    # ec0 K/Q first (dc-outer, 2 psum tiles at a time:
                    # compute paces with the arriving xT DMA chunks)
                    for wsrc, bias_t, dst in ((wks, bk_t, kTs), (wqs, bq_t, qTs)):
                        for sp in range(2):
                            pss = [ppqk.tile([128, 512], DT.float32, tag="ppqk",
                                             name=f"ps0{sb}") for sb in range(2)]
                            for dc in range(NDC):
                                for sb in range(2):
                                    nc.tensor.matmul(
                                        pss[sb][:],
                                        lhsT=wsrc[:, dc, 0:128],
                                        rhs=xTs[:, dc, (sp * 2 + sb) * 512:
                                                (sp * 2 + sb + 1) * 512],
                                        start=(dc == 0), stop=(dc == NDC - 1),
                                    )
                            for sb in range(2):
                                nc.vector.tensor_scalar(
                                    out=dst[:, 0, (sp * 2 + sb) * 512:
                                            (sp * 2 + sb + 1) * 512],
                                    in0=pss[sb][:], scalar1=bias_t[:, 0:1],
                                    scalar2=None, op0=OP.add,
                                )

                    # head-0/qh-0 scores interleaved with the remaining proj work
                    sc0 = make_sc_step(expT0, 0, 0)
                    thunks = []
                    for sb in range(4):
                        thunks.append(lambda sb=sb: qk_proj(wks, bk_t, kTs, 1, sb))
                        thunks.append(lambda sb=sb: qk_proj(wqs, bq_t, qTs, 1, sb))
                    for sc in range(NKT):
                        thunks.append(lambda sc=sc: v_proj(sc))
                    ti = 0
                    for kt in range(NKT):
                        sc0(kt)
                        take = (len(thunks) * (kt + 1)) // NKT - ti
                        for _ in range(take):
                            thunks[ti](); ti += 1

                # ----- attention (phase P pools closed; ctx psum opens) -----
                with tc.tile_pool(name="ps_ctx", bufs=1, space="PSUM") as ps_ctx:
                    for qh in range(QH):
                        qsl = slice(qh * QW, (qh + 1) * QW)
                        acc = pa_acc.tile([128, NKT, QW], DT.float16, tag="acc")
                        for h in range(HPC):
                            hc, hp = h // 2, (h % 2) * 64
                            if qh == 0 and h == 0:
                                expT_h = expT0
                            else:
                                expT_h = pa_exp.tile([128, NKT, QW], DT.float16,
                                                     tag="expT")
                            ctxp = ps_ctx.tile([65, QW], DT.float32, tag="ctxp")

                            sc_step = make_sc_step(expT_h, h, qh)

                            def av_step(kt):
                                for qq in range(2):
                                    nc.tensor.matmul(
                                        ctxp[:, qq * 512:(qq + 1) * 512],
                                        lhsT=vv[:, h, kt, 0:65],
                                        rhs=expT_h[:, kt, qq * 512:(qq + 1) * 512],
                                        start=(kt == 0), stop=(kt == NKT - 1),
                                    )

                            if qh == 0 and h == 0:
                                for kt in range(NKT):
                                    av_step(kt)
                            else:
                                GB = 4
                                for g in range(NKT // GB):
                                    for kt in range(g * GB, (g + 1) * GB):
                                        sc_step(kt)
                                    if g > 0:
                                        for kt in range((g - 1) * GB, g * GB):
                                            av_step(kt)
                                for kt in range(NKT - GB, NKT):
                                    av_step(kt)

                            # per-head Z chain: evac -> compact -> recip -> bcast
                            zal = pa_z.tile([1, QW], DT.float16, tag="zal")
                            nc.vector.tensor_copy(zal[0:1, :], ctxp[64:65, :])
                            ctxu = pa_cu.tile([65, QW], DT.float16, tag="ctxu")
                            nc.vector.tensor_copy(ctxu[:], ctxp[:])
                            zc = pa_z.tile([128, QW // 128], DT.float16, tag="zc")
                            nc.gpsimd.dma_start(out=zc[:], in_=zal[0:1, :])
                            zcf = pa_z.tile([128, QW // 128], DT.float32, tag="zcf")
                            nc.vector.tensor_copy(zcf[:], zc[:])
                            zci = pa_z.tile([128, QW // 128], DT.float32, tag="zci")
                            nc.vector.reciprocal(zci[:], zcf[:])
                            zch = pa_z.tile([128, QW // 128], DT.float16, tag="zch")
                            nc.vector.tensor_copy(zch[:], zci[:])
                            zrow = pa_z.tile([1, QW], DT.float16, tag="zrow")
                            nc.gpsimd.dma_start(out=zrow[0:1, :], in_=zch[:])
                            zbb = pa_zb.tile([128, QW], DT.float16, tag="zbb")
                            nc.gpsimd.partition_broadcast(zbb[:], zrow[0:1, :])

                            nc.vector.tensor_tensor(
                                out=ctxT2[hp:hp + 64, hc, qsl],
                                in0=ctxu[0:64, :], in1=zbb[0:64, :], op=OP.mult,
                            )
                            for kt in range(NKT):
                                if h == 0:
                                    nc.vector.tensor_tensor(
                                        out=acc[:, kt, :], in0=expT_h[:, kt, :],
                                        in1=zbb[:], op=OP.mult,
                                    )
                                else:
                                    tmp = pa_sm.tile([128, QW], DT.float16, tag="tmp")
                                    nc.vector.tensor_tensor(
                                        out=tmp[:], in0=expT_h[:, kt, :], in1=zbb[:],
                                        op=OP.mult,
                                    )
                                    nc.vector.tensor_tensor(
                                        out=acc[:, kt, :], in0=acc[:, kt, :],
                                        in1=tmp[:], op=OP.add,
                                    )
                        for kt in range(NKT):
                            nc.sync.dma_start(
                                out=accT_d[kt * 128:(kt + 1) * 128, qsl],
                                in_=acc[:, kt, :],
                            )

                        # out projection for this s-half
                        for ft in range(D // 128):
                            pos = [ps_ctx.tile([128, 512], DT.float32, tag="pso",
                                               name=f"pso{i}") for i in range(2)]
                            for ec in range(2):
                                for sb in range(2):
                                    nc.tensor.matmul(
                                        pos[sb][:],
                                        lhsT=wos[:, ec, ft * 128:(ft + 1) * 128],
                                        rhs=ctxT2[:, ec, qh * QW + sb * 512:
                                                  qh * QW + (sb + 1) * 512],
                                        start=(ec == 0), stop=(ec == 1),
                                    )
                            for sb in range(2):
                                ot = pa_sm.tile([128, 512], DT.float32, tag="ot")
                                nc.scalar.activation(ot[:], pos[sb][:], AF.Copy)
                                nc.sync.dma_start(
                                    out=outT_d[ft * 128:(ft + 1) * 128,
                                               qh * QW + sb * 512:
                                               qh * QW + (sb + 1) * 512],
                                    in_=ot[:],
                                )

    nc.compile()
    _cache["nc"] = nc
    return nc


def _prep_inputs(x, mask, Wq, bq, Wk, bk, Wv, bv, Wo, bo):
    """Build the 8 per-core input maps (host-side shard + transpose)."""
    x = np.asarray(x, np.float32)
    mask = np.asarray(mask)
    Wq = np.asarray(Wq, np.float32); bq = np.asarray(bq, np.float32)
    Wk = np.asarray(Wk, np.float32); bk = np.asarray(bk, np.float32)
    Wv = np.asarray(Wv, np.float32)
    Wo = np.asarray(Wo, np.float32)

    WqT = (Wq.T / 8.0).astype(np.float16)   # scores scale folded in
    WkT = Wk.T.astype(np.float16)
    WvT = Wv.T.astype(np.float16)
    WoT = Wo.T.astype(np.float16)

    in_maps = []
    for c in range(NCORES):
        b, hg = c // HPC, c % HPC
        esl = slice(hg * ESL, (hg + 1) * ESL)
        mb = (-1e9 * (1.0 - mask[b].astype(np.float32)))
        in_maps.append({
            "xT": np.ascontiguousarray(x[b].T.astype(np.float16)),
            "wq": np.ascontiguousarray(WqT[:, esl]),
            "wk": np.ascontiguousarray(WkT[:, esl]),
            "wv": np.ascontiguousarray(WvT[:, esl]),
            "wo": np.ascontiguousarray(WoT[esl, :]),
            "bq": np.ascontiguousarray((bq[esl] / 8.0).reshape(2, 128).T),
            "bk": np.ascontiguousarray(bk[esl].reshape(2, 128).T),
            "mb": np.ascontiguousarray(mb.reshape(NKT, 128).T),
        })
    return in_maps


def _run(inputs, trace=False):
    nc = _build()
    in_maps = _prep_inputs(**{k: inputs[k] for k in
                              ("x", "mask", "Wq", "bq", "Wk", "bk",
                               "Wv", "bv", "Wo", "bo")})
    res = run_bass_kernel_spmd(nc, in_maps, core_ids=list(range(NCORES)),
                               trace=trace)
    bv = np.asarray(inputs["bv"], np.float32)
    bo = np.asarray(inputs["bo"], np.float32)
    Wo = np.asarray(inputs["Wo"], np.float32)
    corr = bv @ Wo.T + bo   # sum_k p_k = 1 makes bv a constant additive term

    out = np.empty((B, S, D), np.float32)
    attn = np.empty((B, S, S), np.float32)
    for b in range(B):
        outT = np.zeros((D, S), np.float32)
        accT = np.zeros((S, S), np.float32)
        for hg in range(HPC):
            r = res.results[b * HPC + hg]
            outT += r["outT"]
            accT += r["accT"].astype(np.float32)
        out[b] = outT.T + corr
        attn[b] = accT.T / float(H)
    return (out, attn), res


def kernel(**inputs):
    (out, attn), _ = _run(inputs, trace=False)
    return out, attn
